# revision 25
# baseline (speedup 1.0000x reference)
"""Trainium2 Bass kernel for nn_Block_55336358643145 (dense transformer block).

Row-shards the 4096 (batch*seq) rows across 8 NeuronCores: 512 rows per core as
two 256-row blocks (global 256-blocks p and p+4 within the core's batch) so
causal attention work balances. Per core: LN1 -> transpose -> Q/K/V (q,k in
head-transposed layout, v row-layout with a per-head ones column so the softmax
denominator falls out of the attn@v matmul), AllGather of k/v within each
4-core batch group, kpos-major masked softmax (per-partition bias-column masks;
exp of fully-masked/padded tiles is exactly 0, giving one uniform SPMD program
for all cores), attn@v, out-proj + residual, LN2, 4x MLP with exact-erf Gelu.
All matmuls run in float32r (full PE rate, ~2e-4 rel err). Host reassembles.
"""

import contextlib

import numpy as np

import concourse.bass as bass
import concourse.tile as tile
from concourse import bacc, mybir
from concourse.bass_utils import run_bass_kernel_spmd

F32 = mybir.dt.float32
F32R = mybir.dt.float32r
AF = mybir.ActivationFunctionType
ALU = mybir.AluOpType

B, S, D, H, HD, FF = 2, 2048, 1024, 16, 64, 4096
NCORE = 8
R = 512            # rows per core
QB = 256           # q rows per block
NBLK = 2           # blocks per core
KTW = 128          # kpos tile width
NB_PAD = (14, 14)  # pass-A padded tile counts per block
LN_EPS = 1e-5
JD = 25            # joined dim for the column-zero mask
NEG = -1.0e30
DC = D // 128      # 8 d-chunks
GC = FF // 128     # 32 mlp hidden chunks
VW = H * (HD + 1)  # 1040: v with per-head ones column


BF16 = mybir.dt.bfloat16


def _gtile_src(b, t):
    """(block b, 128-wide kpos tile t) -> (rank, col/row offset) in gather."""
    p = t // 2
    if b == 0:
        return p, 128 * (t % 2)          # batch 0, position p -> rank p
    return 7 - p, 256 + 128 * (t % 2)    # batch 1, position p -> rank 7-p


def build_program(apply_bv, apply_ln1_gb, apply_ln2_gb):
    nc = bacc.Bacc("TRN2", target_bir_lowering=False, debug=False,
                   num_devices=NCORE)

    def inp(name, shape):
        return nc.dram_tensor(name, list(shape), F32, kind="ExternalInput").ap()

    def binp(name, shape):
        return nc.dram_tensor(name, list(shape), BF16,
                              kind="ExternalInput").ap()

    io = dict(
        hs=inp("hs", (R, D)),
        wq=binp("wq", (D, D)), wk=binp("wk", (D, D)),
        wv=binp("wv", (D, D)), wp=binp("wp", (D, D)),
        w1=binp("w1", (GC, 128, DC, 128)), w2=binp("w2", (FF, D)),
        bq8=inp("bq8", (128, DC)), bkl=inp("bkl", (128, DC)),
        bvh=inp("bvh", (HD, H)), b1l=inp("b1l", (128, GC)),
        bpr=binp("bpr", (1, D)), b2r=binp("b2r", (1, D)),
        ln1gb=inp("ln1gb", (2, D)), ln2gb=inp("ln2gb", (2, D)),
        biasA=inp("biasA", (128, NBLK, max(NB_PAD))),
        biasB=inp("biasB", (128, NBLK, 2)),
        maskAB=inp("maskAB", (2, 128, 2 * KTW)),
        ident=inp("ident", (128, 128)),
        onesr=binp("onesr", (1, 128)),
        vones=nc.dram_tensor("vones", [128, H, 1], BF16, kind="ExternalInput").ap(),
        out=nc.dram_tensor("out", [R, D], F32, kind="ExternalOutput").ap(),
    )

    with tile.TileContext(nc) as tc:
        _build(tc, io, apply_bv, apply_ln1_gb, apply_ln2_gb)
    nc.compile()
    return nc


def _build(tc, io, apply_bv, apply_ln1_gb, apply_ln2_gb):
    nc = tc.nc
    hs, out = io["hs"], io["out"]

    with contextlib.ExitStack() as ctx:
        persist = ctx.enter_context(tc.tile_pool(name="persist", bufs=1, side="left"))
        dram = ctx.enter_context(tc.tile_pool(name="dram", bufs=1,
                                              space="DRAM"))

        # ---- small constants ------------------------------------------------
        ident_sb = persist.tile([128, 128], F32)
        nc.sync.dma_start(ident_sb[:], io["ident"][:])
        eps_sb = persist.tile([128, 1], F32)
        nc.vector.memset(eps_sb[:], LN_EPS)
        ones_r = persist.tile([1, 128], BF16)
        nc.sync.dma_start(ones_r[:], io["onesr"][:])
        bq8_sb = persist.tile([128, DC], F32)
        nc.sync.dma_start(bq8_sb[:], io["bq8"][:])
        bkl_sb = persist.tile([128, DC], F32)
        nc.sync.dma_start(bkl_sb[:], io["bkl"][:])
        b1l_sb = persist.tile([128, GC], F32)
        nc.sync.dma_start(b1l_sb[:], io["b1l"][:])
        bpr_sb = persist.tile([1, D], BF16)
        nc.sync.dma_start(bpr_sb[:], io["bpr"][:])
        b2r_sb = persist.tile([1, D], BF16)
        nc.sync.dma_start(b2r_sb[:], io["b2r"][:])
        biasA_sb = persist.tile([128, NBLK, max(NB_PAD)], F32)
        nc.sync.dma_start(biasA_sb[:], io["biasA"][:])
        biasB_sb = persist.tile([128, NBLK, 2], F32)
        nc.sync.dma_start(biasB_sb[:], io["biasB"][:])
        maskA_sb = persist.tile([128, 2, 2 * KTW], F32)
        for j in range(2):
            nc.sync.dma_start(maskA_sb[:, j, :], io["maskAB"][0])
        maskA_sb = maskA_sb[:].rearrange("p a b -> p (a b)")
        maskB_sb = persist.tile([128, 2, 2 * KTW], F32)
        for j in range(2):
            nc.sync.dma_start(maskB_sb[:, j, :], io["maskAB"][1])
        maskB_sb = maskB_sb[:].rearrange("p a b -> p (a b)")
        if apply_bv:
            bvh_sb = persist.tile([HD, H], F32)
            nc.sync.dma_start(bvh_sb[:], io["bvh"][:])

        def ln_gb_tiles(gb_inp, nm):
            g_sb = persist.tile([128, D], F32, name=f"g_{nm}")
            b_sb = persist.tile([128, D], F32, name=f"b_{nm}")
            g_row = persist.tile([1, D], F32, name=f"gr_{nm}")
            b_row = persist.tile([1, D], F32, name=f"br_{nm}")
            nc.sync.dma_start(g_row[:], gb_inp[0:1, :])
            nc.sync.dma_start(b_row[:], gb_inp[1:2, :])
            nc.gpsimd.partition_broadcast(g_sb[:], g_row[:])
            nc.gpsimd.partition_broadcast(b_sb[:], b_row[:])
            return g_sb, b_sb

        ln1_g = ln1_b = ln2_g = ln2_b = None
        if apply_ln1_gb:
            ln1_g, ln1_b = ln_gb_tiles(io["ln1gb"], "ln1")
        if apply_ln2_gb:
            ln2_g, ln2_b = ln_gb_tiles(io["ln2gb"], "ln2")

        def layernorm(dst, src, pool, g_sb, b_sb):
            stats = pool.tile([128, 2, 6], F32, tag="ln_stats")
            sg = src.rearrange("p (g d) -> p g d", g=2)
            for g in range(2):
                nc.vector.bn_stats(out=stats[:, g, :], in_=sg[:, g, :])
            mv = pool.tile([128, 2], F32, tag="ln_mv")
            nc.vector.bn_aggr(out=mv[:], in_=stats[:])
            rstd = pool.tile([128, 1], F32, tag="ln_rstd")
            nc.scalar.activation(out=rstd[:], in_=mv[:, 1:2], func=AF.Sqrt,
                                 bias=eps_sb[:], scale=1.0)
            nc.vector.reciprocal(out=rstd[:], in_=rstd[:])
            nc.vector.tensor_scalar(out=dst, in0=src, scalar1=mv[:, 0:1],
                                    scalar2=rstd[:], op0=ALU.subtract,
                                    op1=ALU.mult)
            if g_sb is not None:
                nc.vector.tensor_mul(dst, dst, g_sb[:])
                nc.vector.tensor_add(dst, dst, b_sb[:])

        def transpose_into(dstT, src_tile, rt, tp_pool):
            for c in range(DC):
                tp = tp_pool.tile([128, 128], F32, tag="tp")
                nc.tensor.transpose(tp[:], src_tile[:, 128 * c:128 * (c + 1)],
                                    ident_sb[:])
                nc.scalar.copy(dstT[:, c, 128 * rt:128 * (rt + 1)], tp[:])

        def proj_headT(dstT, w_inp, bias_sb, scale, nm):
            """dstT[:, oc, :] = ((x @ w) * scale + bias)^T rows 128oc..+128."""
            with tc.tile_pool(name=f"w_{nm}", bufs=1, side="left") as wpl, \
                 tc.tile_pool(name=f"ps_{nm}", bufs=2, space="PSUM") as pps:
                wts = []
                for c in range(DC):
                    wt = wpl.tile([128, D], BF16, tag=f"w{c}",
                                  name=f"w_{nm}_{c}")
                    nc.sync.dma_start(
                        wt[:], w_inp[128 * c:128 * (c + 1), :])
                    wts.append(wt)
                for oc in range(DC):
                    ps = pps.tile([128, R], F32, tag="ps", name=f"ps_{nm}_{oc}")
                    for c in range(DC):
                        nc.tensor.matmul(
                            ps[:], wts[c][:, 128 * oc:128 * (oc + 1)],
                            xT[:, c, :], start=(c == 0), stop=(c == DC - 1))
                    nc.scalar.activation(dstT[:, oc, :], ps[:],
                                         func=AF.Identity,
                                         bias=bias_sb[:, oc:oc + 1],
                                         scale=scale)

        # DRAM bounce + gathered buffers for the k/v AllGathers
        KVN = D * R + R * VW
        kv_loc = dram.tile([KVN], BF16)
        kv_g = dram.tile([8, KVN], BF16, addr_space="Shared")

        def k_view(ap1d):
            return ap1d[0:D * R].rearrange("(c p q) -> p c q", p=128, q=R)

        def v_view(ap1d):
            return ap1d[D * R:D * R + R * VW].rearrange("(r w) -> r w", w=VW)

        es_x = ctx.enter_context(contextlib.ExitStack())      # xT: P0..P3
        es_q = ctx.enter_context(contextlib.ExitStack())      # qT: P3..P4
        xT_pool = es_x.enter_context(
            tc.tile_pool(name="xT_p", bufs=1, side="left"))
        qT_pool = es_q.enter_context(
            tc.tile_pool(name="qT_p", bufs=1, side="right"))
        xT = xT_pool.tile([128, DC, R], BF16)
        qT = qT_pool.tile([128, DC, R], BF16)

        # ================= P0: load + LN1 + transpose ========================
        with tc.tile_pool(name="p0", bufs=2, side="left") as p0, \
             tc.tile_pool(name="p0ps", bufs=4, space="PSUM") as p0ps:
            for rt in range(4):
                hst = p0.tile([128, D], F32, tag="hst")
                nc.sync.dma_start(hst[:], hs[128 * rt:128 * (rt + 1), :])
                xln = p0.tile([128, D], F32, tag="xln")
                layernorm(xln[:], hst[:], p0, ln1_g, ln1_b)
                transpose_into(xT, xln, rt, p0ps)

        # ================= P1: kT; AllGather(k) ==============================
        with tc.tile_pool(name="kT_p", bufs=1, side="right") as kT_pool:
            kT = kT_pool.tile([128, DC, R], BF16)
            proj_headT(kT, io["wk"], bkl_sb, 1.0, "wk")
            k_loc_v = k_view(kv_loc)
            for c in range(DC):
                nc.sync.dma_start(k_loc_v[:, c, :], kT[:, c, :].bitcast(BF16))

        # ================= P2: v rows + ones cols; AllGather(v) ==============
        with tc.tile_pool(name="vaug_p", bufs=1, side="right") as vaug_pool, \
             tc.tile_pool(name="w_wv", bufs=1, side="left") as wpl, \
             tc.tile_pool(name="ps_wv", bufs=2, space="PSUM") as pps:
            vaug = vaug_pool.tile([128, 4, VW], BF16)
            wts = []
            for c in range(DC):
                wt = wpl.tile([128, D], BF16, tag=f"w{c}", name=f"w_wv_{c}")
                nc.sync.dma_start(
                    wt[:], io["wv"][128 * c:128 * (c + 1), :])
                wts.append(wt)
            for pt in range(4):
                for cg in range(2):
                    ps = pps.tile([128, 512], F32, tag="ps",
                                  name=f"ps_wv_{pt}_{cg}")
                    for c in range(DC):
                        nc.tensor.matmul(
                            ps[:], xT[:, c, 128 * pt:128 * (pt + 1)],
                            wts[c][:, 512 * cg:512 * (cg + 1)],
                            start=(c == 0), stop=(c == DC - 1))
                    for hh in range(8):
                        h = 8 * cg + hh
                        nc.scalar.copy(
                            vaug[:, pt, (HD + 1) * h:(HD + 1) * h + HD],
                            ps[:, HD * hh:HD * (hh + 1)])
                nc.sync.dma_start(
                    vaug[:, pt, :].rearrange("p (h e) -> p h e", e=HD + 1)
                    [:, :, HD:HD + 1],
                    io["vones"][:])
                nc.sync.dma_start(
                    v_view(kv_loc)[128 * pt:128 * (pt + 1), :],
                    vaug[:, pt, :])
        nc.gpsimd.collective_compute(
            "AllGather", ALU.bypass,
            replica_groups=[[0, 1, 2, 3, 4, 5, 6, 7]],
            ins=[kv_loc.opt()], outs=[kv_g.opt()])

        # ================= P3: qT (scaled by 1/8) ============================
        proj_headT(qT, io["wq"], bq8_sb, 0.125, "wq")
        es_x.close()  # xT no longer needed
        assert qT is not None

        # ================= P4: attention =====================================
        es_attn = ctx.enter_context(contextlib.ExitStack())   # attn_oT: P4..P5
        ao_pool = es_attn.enter_context(tc.tile_pool(name="ao_p", bufs=1, side="left"))
        attn_oT = [ao_pool.tile([128, DC, QB], BF16, name=f"attn_oT{b}")
                   for b in range(NBLK)]
        with tc.tile_pool(name="kg_pool", bufs=1, side="left") as kgp, \
             tc.tile_pool(name="vg_pool", bufs=1, side="left") as vgp, \
             tc.tile_pool(name="own_pool", bufs=2, side="left") as ownp, \
             tc.tile_pool(name="at_sb", bufs=4, side="left") as asb, \
             tc.tile_pool(name="at_norm", bufs=2, side="left") as anorm, \
             tc.tile_pool(name="sc_ps", bufs=2, space="PSUM") as scps, \
             tc.tile_pool(name="oT_ps", bufs=4, space="PSUM") as otps:
            kranks, vranks = [], []
            for r in range(8):
                kr = kgp.tile([128, DC, R], BF16, tag=f"kr{r}", name=f"kr_{r}")
                nc.sync.dma_start(kr[:], k_view(kv_g[r]))
                kranks.append(kr)
                vr = []
                for pt in range(4):
                    vt = vgp.tile([128, VW], BF16, tag=f"vr{r}_{pt}",
                                  name=f"vr_{r}_{pt}")
                    nc.sync.dma_start(vt[:], v_view(kv_g[r])[128 * pt:128 * (pt + 1), :])
                    vr.append(vt)
                vranks.append(vr)

            def ktile_ap(b, t, hp, hc):
                r, off = _gtile_src(b, t)
                return kranks[r][hp, hc, off:off + KTW]

            def vtile_ap(b, t, vs):
                r, off = _gtile_src(b, t)
                return vranks[r][off // 128][:, vs]

            k_loc_v2 = k_view(kv_loc)
            for b in range(NBLK):
                qs = slice(QB * b, QB * (b + 1))
                kown, vown = [], []
                for i in range(2):
                    sl = 2 * b + i
                    ko = ownp.tile([128, DC, KTW], BF16, tag="kown",
                                   name=f"kown_{b}_{i}")
                    nc.sync.dma_start(
                        ko[:], k_loc_v2[:, :, KTW * sl:KTW * (sl + 1)])
                    kown.append(ko)
                    vo = ownp.tile([128, VW], BF16, tag="vown",
                                   name=f"vown_{b}_{i}")
                    nc.sync.dma_start(
                        vo[:], v_view(kv_loc)[KTW * sl:KTW * (sl + 1), :])
                    vown.append(vo)
                for hh in range(H // 2):
                    h0, h1 = 2 * hh, 2 * hh + 1
                    hps = (slice(0, 64), slice(64, 128))
                    vss = (slice((HD + 1) * h0, (HD + 1) * (h0 + 1)),
                           slice((HD + 1) * h1, (HD + 1) * (h1 + 1)))
                    oTs = [otps.tile([HD + 1, QB], F32, tag="oT",
                                     name=f"oT_{b}_{h0}_{j}")
                           for j in range(2)]
                    # pass B first: own (diagonal) kpos tiles, indep of the AG
                    for i in range(2):
                        sc = scps.tile([128, 2, 512], F32, tag="sc",
                                       name=f"scB_{b}_{hh}_{i}")
                        scv = sc[:, :, 0:QB]
                        for j in range(2):
                            nc.tensor.matmul(sc[:, j, 0:QB],
                                             kown[i][hps[j], hh, :],
                                             qT[hps[j], hh, qs],
                                             start=True, stop=True)
                        m_sb = maskA_sb if i == 0 else maskB_sb
                        nc.vector.tensor_add(
                            scv, scv,
                            m_sb.rearrange("p (a b) -> p a b", a=2))
                        ex = asb.tile([128, 2, QB], BF16, tag="ex",
                                      name=f"exB_{b}_{hh}_{i}")
                        nc.scalar.activation(ex[:], scv, func=AF.Exp,
                                             bias=biasB_sb[:, b, i:i + 1],
                                             scale=1.0)
                        for j in range(2):
                            nc.tensor.matmul(oTs[j][:], vown[i][:, vss[j]],
                                             ex[:, j, :],
                                             start=(i == 0), stop=False)
                    # pass A: full (or padded-out) gathered kpos tiles
                    for t in range(NB_PAD[b]):
                        sc = scps.tile([128, 2, 512], F32, tag="sc",
                                       name=f"scA_{b}_{hh}_{t}")
                        for j in range(2):
                            nc.tensor.matmul(sc[:, j, 0:QB],
                                             ktile_ap(b, t, hps[j], hh),
                                             qT[hps[j], hh, qs],
                                             start=True, stop=True)
                        ex = asb.tile([128, 2, QB], BF16, tag="ex",
                                      name=f"exA_{b}_{hh}_{t}")
                        nc.scalar.activation(ex[:], sc[:, :, 0:QB],
                                             func=AF.Exp,
                                             bias=biasA_sb[:, b, t:t + 1],
                                             scale=1.0)
                        for j in range(2):
                            nc.tensor.matmul(oTs[j][:],
                                             vtile_ap(b, t, vss[j]),
                                             ex[:, j, :],
                                             start=False,
                                             stop=(t == NB_PAD[b] - 1))
                    # normalize by the ones-column denominator (psum row HD)
                    for j, h in enumerate((h0, h1)):
                        oT = oTs[j]
                        rec = anorm.tile([1, QB], F32, tag="rec",
                                         name=f"rec_{b}_{h}")
                        nc.vector.reciprocal(rec[:], oT[HD:HD + 1, :])
                        rb = anorm.tile([64, QB], F32, tag="rb",
                                        name=f"rb_{b}_{h}")
                        nc.gpsimd.partition_broadcast(rb[:], rec[:])
                        if j == 0:
                            dst = attn_oT[b][0:HD, hh, :]
                            nc.vector.tensor_mul(dst, oT[0:HD, :], rb[:])
                            if apply_bv:
                                nc.vector.tensor_scalar_add(
                                    dst, dst, bvh_sb[:, h:h + 1])
                        else:
                            tmpn = anorm.tile([64, QB], BF16, tag="tmpn",
                                              name=f"tmpn_{b}_{h}")
                            nc.vector.tensor_mul(tmpn[:], oT[0:HD, :], rb[:])
                            if apply_bv:
                                nc.vector.tensor_scalar_add(
                                    tmpn[:], tmpn[:], bvh_sb[:, h:h + 1])
                            nc.sync.dma_start(attn_oT[b][64:128, hh, :],
                                              tmpn[:])
        es_q.close()  # qT done

        # ================= P5: out-proj + residual ===========================
        es_h = ctx.enter_context(contextlib.ExitStack())      # h_sb: P5..P8
        h_pool = es_h.enter_context(tc.tile_pool(name="h_p", bufs=1, side="right"))
        h_sb = h_pool.tile([128, 4, D], F32)
        with tc.tile_pool(name="w_wp", bufs=1, side="left") as wpl, \
             tc.tile_pool(name="hs2", bufs=2, side="left") as hs2, \
             tc.tile_pool(name="ps_wp", bufs=2, space="PSUM") as pps:
            wts = []
            for c in range(DC):
                wt = wpl.tile([128, D], BF16, tag=f"w{c}", name=f"w_wp_{c}")
                nc.sync.dma_start(
                    wt[:], io["wp"][128 * c:128 * (c + 1), :])
                wts.append(wt)
            for rt in range(4):
                b, qt = rt // 2, rt % 2
                hst = hs2.tile([128, D], F32, tag="hst", name=f"hst_{rt}")
                nc.sync.dma_start(hst[:], hs[128 * rt:128 * (rt + 1), :])
                for cg in range(2):
                    ps = pps.tile([128, 512], F32, tag="ps",
                                  name=f"ps_wp_{rt}_{cg}")
                    for c in range(DC):
                        nc.tensor.matmul(
                            ps[:], attn_oT[b][:, c, 128 * qt:128 * (qt + 1)],
                            wts[c][:, 512 * cg:512 * (cg + 1)],
                            start=(c == 0), stop=False)
                    nc.tensor.matmul(ps[:], ones_r[:],
                                     bpr_sb[:, 512 * cg:512 * (cg + 1)],
                                     start=False, stop=True)
                    nc.vector.tensor_add(h_sb[:, rt, 512 * cg:512 * (cg + 1)],
                                         ps[:], hst[:, 512 * cg:512 * (cg + 1)])
        es_attn.close()  # attn_oT done

        # ================= P6: LN2 + transpose ===============================
        es_mlp = ctx.enter_context(contextlib.ExitStack())    # h2T, gT
        mlp_pool = es_mlp.enter_context(tc.tile_pool(name="mlp_p", bufs=1, side="left"))
        h2T = mlp_pool.tile([128, DC, R], BF16)
        gT = mlp_pool.tile([128, GC, R], BF16)
        with tc.tile_pool(name="p6", bufs=2, side="left") as p6, \
             tc.tile_pool(name="p6ps", bufs=4, space="PSUM") as p6ps:
            for rt in range(4):
                h2 = p6.tile([128, D], F32, tag="h2")
                layernorm(h2[:], h_sb[:, rt, :], p6, ln2_g, ln2_b)
                transpose_into(h2T, h2, rt, p6ps)

        # ================= P7: MLP up + gelu =================================
        with tc.tile_pool(name="w_w1", bufs=3, side="left") as wpl, \
             tc.tile_pool(name="ps_w1", bufs=2, space="PSUM") as pps:
            for gc in range(GC):
                wt = wpl.tile([128, DC, 128], BF16, tag="w1")
                nc.sync.dma_start(wt[:], io["w1"][gc])
                ps = pps.tile([128, R], F32, tag="ps", name=f"ps_w1_{gc}")
                for c in range(DC):
                    nc.tensor.matmul(ps[:], wt[:, c, :], h2T[:, c, :],
                                     start=(c == 0), stop=(c == DC - 1))
                nc.scalar.activation(gT[:, gc, :], ps[:], func=AF.Gelu,
                                     bias=b1l_sb[:, gc:gc + 1], scale=1.0)

        # ================= P8: MLP down + bias + residual ====================
        with tc.tile_pool(name="w_w2", bufs=3, side="left") as wpl, \
             tc.tile_pool(name="o_sb", bufs=2, side="left") as osb, \
             tc.tile_pool(name="o_ps", bufs=1, space="PSUM") as pps:
            psts = [pps.tile([128, 512], F32, tag=f"o{i}", name=f"o_ps_{i}")
                    for i in range(8)]
            for gc in range(GC):
                wt = wpl.tile([128, D], BF16, tag="w2")
                nc.sync.dma_start(
                    wt[:], io["w2"][128 * gc:128 * (gc + 1), :])
                for qt in range(4):
                    for cg in range(2):
                        nc.tensor.matmul(
                            psts[2 * qt + cg][:],
                            gT[:, gc, 128 * qt:128 * (qt + 1)],
                            wt[:, 512 * cg:512 * (cg + 1)],
                            start=(gc == 0), stop=False)
            for qt in range(4):
                ot = osb.tile([128, D], F32, tag="ot", name=f"ot_{qt}")
                for cg in range(2):
                    nc.tensor.matmul(psts[2 * qt + cg][:], ones_r[:],
                                     b2r_sb[:, 512 * cg:512 * (cg + 1)],
                                     start=False, stop=True)
                    nc.vector.tensor_add(ot[:, 512 * cg:512 * (cg + 1)],
                                         psts[2 * qt + cg][:],
                                         h_sb[:, qt, 512 * cg:512 * (cg + 1)])
                nc.sync.dma_start(out[128 * qt:128 * (qt + 1), :], ot[:])


# ---------------------------------------------------------------------------
# Host side
# ---------------------------------------------------------------------------

_CACHE = {}
LAST_RESULT = None  # BassKernelResults of the most recent run (for test.py)


def _get_program(key):
    if key not in _CACHE:
        _CACHE[key] = build_program(*key)
    return _CACHE[key]


def _colzero_bias(kpos):
    return np.where((kpos % JD) == (JD - 1), np.float32(NEG), np.float32(0.0))


def kernel(hidden_states, Wq, bq, Wk, bk, Wv, bv, Wp, bp,
           ln1_g, ln1_b, ln2_g, ln2_b, W1, b1, W2, b2):
    f32 = lambda a: np.ascontiguousarray(np.asarray(a, dtype=np.float32))
    hidden_states = f32(hidden_states)
    Wq, bq, Wk, bk, Wv, bv, Wp, bp = map(f32, (Wq, bq, Wk, bk, Wv, bv, Wp, bp))
    ln1_g, ln1_b, ln2_g, ln2_b = map(f32, (ln1_g, ln1_b, ln2_g, ln2_b))
    W1, b1, W2, b2 = map(f32, (W1, b1, W2, b2))

    apply_bv = bool(np.any(bv != 0.0))
    apply_ln1 = bool(np.any(ln1_g != 1.0) or np.any(ln1_b != 0.0))
    apply_ln2 = bool(np.any(ln2_g != 1.0) or np.any(ln2_b != 0.0))
    nc = _get_program((apply_bv, apply_ln1, apply_ln2))

    chunk_major = lambda v: np.ascontiguousarray(v.reshape(-1, 128).T)
    kp = np.arange(KTW)[:, None]
    iq = np.arange(KTW)[None, :]
    tri = np.where(kp <= iq, np.float32(0.0), np.float32(NEG))
    maskAB = np.zeros((2, 128, 2 * KTW), dtype=np.float32)
    maskAB[0, :, :KTW] = tri
    maskAB[1, :, :KTW] = NEG
    maskAB[1, :, KTW:] = tri

    import ml_dtypes
    bf = lambda a: np.ascontiguousarray(a.astype(ml_dtypes.bfloat16))
    w1x = np.ascontiguousarray(
        W1.reshape(DC, 128, GC, 128).transpose(2, 1, 0, 3))
    shared = dict(wq=bf(Wq), wk=bf(Wk), wv=bf(Wv), wp=bf(Wp), w1=bf(w1x),
                  w2=bf(W2),
                  bq8=chunk_major(bq * 0.125), bkl=chunk_major(bk),
                  bvh=np.ascontiguousarray(bv.reshape(H, HD).T),
                  b1l=chunk_major(b1), bpr=bf(bp.reshape(1, D)),
                  b2r=bf(b2.reshape(1, D)), ln1gb=np.stack([ln1_g, ln1_b]),
                  ln2gb=np.stack([ln2_g, ln2_b]), maskAB=maskAB,
                  ident=np.eye(128, dtype=np.float32),
                  onesr=np.ones((1, 128), dtype=ml_dtypes.bfloat16),
                  vones=np.ones((128, H, 1), dtype=ml_dtypes.bfloat16))

    in_maps, row_map = [], []
    for core in range(NCORE):
        # block0 = (batch 0, position core); block1 = (batch 1, pos 7-core)
        positions = (core, 7 - core)
        rows = [np.arange(QB * pb, QB * (pb + 1)) for pb in positions]
        row_map.append(rows)

        biasA = np.full((128, NBLK, max(NB_PAD)), NEG, dtype=np.float32)
        biasB = np.zeros((128, NBLK, 2), dtype=np.float32)
        for b, pb in enumerate(positions):
            for t in range(NB_PAD[b]):
                if t < 2 * pb:
                    biasA[:, b, t] = _colzero_bias(KTW * t + np.arange(KTW))
            for i in range(2):
                biasB[:, b, i] = _colzero_bias(QB * pb + KTW * i
                                               + np.arange(KTW))

        m = dict(shared)
        m["hs"] = np.ascontiguousarray(
            np.concatenate([hidden_states[0, rows[0], :],
                            hidden_states[1, rows[1], :]]))
        m["biasA"] = np.ascontiguousarray(biasA)
        m["biasB"] = np.ascontiguousarray(biasB)
        in_maps.append(m)

    res = run_bass_kernel_spmd(nc, in_maps, core_ids=list(range(NCORE)))
    global LAST_RESULT
    LAST_RESULT = res

    out_full = np.empty((B, S, D), dtype=np.float32)
    for core in range(NCORE):
        rows = row_map[core]
        o = res.results[core]["out"]
        out_full[0, rows[0], :] = o[:QB]
        out_full[1, rows[1], :] = o[QB:]
    return out_full


# revision 26
# speedup vs baseline: 1.0704x; 1.0704x over previous
"""Trainium2 Bass kernel for nn_Block_55336358643145 (dense transformer block).

Row-shards the 4096 (batch*seq) rows across 8 NeuronCores: 512 rows per core as
two 256-row blocks (global 256-blocks p and p+4 within the core's batch) so
causal attention work balances. Per core: LN1 -> transpose -> Q/K/V (q,k in
head-transposed layout, v row-layout with a per-head ones column so the softmax
denominator falls out of the attn@v matmul), AllGather of k/v within each
4-core batch group, kpos-major masked softmax (per-partition bias-column masks;
exp of fully-masked/padded tiles is exactly 0, giving one uniform SPMD program
for all cores), attn@v, out-proj + residual, LN2, 4x MLP with exact-erf Gelu.
All matmuls run in float32r (full PE rate, ~2e-4 rel err). Host reassembles.
"""

import contextlib

import numpy as np

import concourse.bass as bass
import concourse.tile as tile
from concourse import bacc, mybir
from concourse.bass_utils import run_bass_kernel_spmd

F32 = mybir.dt.float32
F32R = mybir.dt.float32r
AF = mybir.ActivationFunctionType
ALU = mybir.AluOpType

B, S, D, H, HD, FF = 2, 2048, 1024, 16, 64, 4096
NCORE = 8
R = 512            # rows per core
QB = 256           # q rows per block
NBLK = 2           # blocks per core
KTW = 128          # kpos tile width
NB_PAD = (14, 14)  # pass-A padded tile counts per block
LN_EPS = 1e-5
JD = 25            # joined dim for the column-zero mask
NEG = -1.0e30
DC = D // 128      # 8 d-chunks
GC = FF // 128     # 32 mlp hidden chunks
VW = H * (HD + 1)  # 1040: v with per-head ones column


BF16 = mybir.dt.bfloat16


def _gtile_src(b, t):
    """(block b, 128-wide kpos tile t) -> (rank, col/row offset) in gather."""
    p = t // 2
    if b == 0:
        return p, 128 * (t % 2)          # batch 0, position p -> rank p
    return 7 - p, 256 + 128 * (t % 2)    # batch 1, position p -> rank 7-p


def build_program(apply_bv, apply_ln1_gb, apply_ln2_gb):
    nc = bacc.Bacc("TRN2", target_bir_lowering=False, debug=False,
                   num_devices=NCORE)

    def inp(name, shape):
        return nc.dram_tensor(name, list(shape), F32, kind="ExternalInput").ap()

    def binp(name, shape):
        return nc.dram_tensor(name, list(shape), BF16,
                              kind="ExternalInput").ap()

    io = dict(
        hs=inp("hs", (R, D)),
        wq=binp("wq", (D, D)), wk=binp("wk", (D, D)),
        wv=binp("wv", (D, D)), wp=binp("wp", (D, D)),
        w1=binp("w1", (GC, 128, DC, 128)), w2=binp("w2", (FF, D)),
        bq8=inp("bq8", (128, DC)), bkl=inp("bkl", (128, DC)),
        bvh=inp("bvh", (HD, H)), b1l=inp("b1l", (128, GC)),
        bpr=binp("bpr", (1, D)), b2r=binp("b2r", (1, D)),
        ln1gb=inp("ln1gb", (2, D)), ln2gb=inp("ln2gb", (2, D)),
        biasA=inp("biasA", (128, NBLK, max(NB_PAD))),
        biasB=inp("biasB", (128, NBLK, 2)),
        maskAB=inp("maskAB", (2, 128, 2 * KTW)),
        ident=inp("ident", (128, 128)),
        onesr=binp("onesr", (1, 128)),
        vones=nc.dram_tensor("vones", [128, H, 1], BF16, kind="ExternalInput").ap(),
        out=nc.dram_tensor("out", [R, D], F32, kind="ExternalOutput").ap(),
    )

    with tile.TileContext(nc) as tc:
        _build(tc, io, apply_bv, apply_ln1_gb, apply_ln2_gb)
    nc.compile()
    return nc


def _build(tc, io, apply_bv, apply_ln1_gb, apply_ln2_gb):
    nc = tc.nc
    hs, out = io["hs"], io["out"]

    with contextlib.ExitStack() as ctx:
        persist = ctx.enter_context(tc.tile_pool(name="persist", bufs=1, side="left"))
        dram = ctx.enter_context(tc.tile_pool(name="dram", bufs=1,
                                              space="DRAM"))

        # ---- small constants ------------------------------------------------
        ident_sb = persist.tile([128, 128], F32)
        nc.sync.dma_start(ident_sb[:], io["ident"][:])
        eps_sb = persist.tile([128, 1], F32)
        nc.vector.memset(eps_sb[:], LN_EPS)
        ones_r = persist.tile([1, 128], BF16)
        nc.sync.dma_start(ones_r[:], io["onesr"][:])
        bq8_sb = persist.tile([128, DC], F32)
        nc.sync.dma_start(bq8_sb[:], io["bq8"][:])
        bkl_sb = persist.tile([128, DC], F32)
        nc.sync.dma_start(bkl_sb[:], io["bkl"][:])
        b1l_sb = persist.tile([128, GC], F32)
        nc.sync.dma_start(b1l_sb[:], io["b1l"][:])
        bpr_sb = persist.tile([1, D], BF16)
        nc.sync.dma_start(bpr_sb[:], io["bpr"][:])
        b2r_sb = persist.tile([1, D], BF16)
        nc.sync.dma_start(b2r_sb[:], io["b2r"][:])
        biasA_sb = persist.tile([128, NBLK, max(NB_PAD)], F32)
        nc.sync.dma_start(biasA_sb[:], io["biasA"][:])
        biasB_sb = persist.tile([128, NBLK, 2], F32)
        nc.sync.dma_start(biasB_sb[:], io["biasB"][:])
        maskA_sb = persist.tile([128, 2, 2 * KTW], F32)
        for j in range(2):
            nc.sync.dma_start(maskA_sb[:, j, :], io["maskAB"][0])
        maskA_sb = maskA_sb[:].rearrange("p a b -> p (a b)")
        maskB_sb = persist.tile([128, 2, 2 * KTW], F32)
        for j in range(2):
            nc.sync.dma_start(maskB_sb[:, j, :], io["maskAB"][1])
        maskB_sb = maskB_sb[:].rearrange("p a b -> p (a b)")
        if apply_bv:
            bvh_sb = persist.tile([HD, H], F32)
            nc.sync.dma_start(bvh_sb[:], io["bvh"][:])

        def ln_gb_tiles(gb_inp, nm):
            g_sb = persist.tile([128, D], F32, name=f"g_{nm}")
            b_sb = persist.tile([128, D], F32, name=f"b_{nm}")
            g_row = persist.tile([1, D], F32, name=f"gr_{nm}")
            b_row = persist.tile([1, D], F32, name=f"br_{nm}")
            nc.sync.dma_start(g_row[:], gb_inp[0:1, :])
            nc.sync.dma_start(b_row[:], gb_inp[1:2, :])
            nc.gpsimd.partition_broadcast(g_sb[:], g_row[:])
            nc.gpsimd.partition_broadcast(b_sb[:], b_row[:])
            return g_sb, b_sb

        ln1_g = ln1_b = ln2_g = ln2_b = None
        if apply_ln1_gb:
            ln1_g, ln1_b = ln_gb_tiles(io["ln1gb"], "ln1")
        if apply_ln2_gb:
            ln2_g, ln2_b = ln_gb_tiles(io["ln2gb"], "ln2")

        def layernorm(dst, src, pool, g_sb, b_sb):
            stats = pool.tile([128, 2, 6], F32, tag="ln_stats")
            sg = src.rearrange("p (g d) -> p g d", g=2)
            for g in range(2):
                nc.vector.bn_stats(out=stats[:, g, :], in_=sg[:, g, :])
            mv = pool.tile([128, 2], F32, tag="ln_mv")
            nc.vector.bn_aggr(out=mv[:], in_=stats[:])
            rstd = pool.tile([128, 1], F32, tag="ln_rstd")
            nc.scalar.activation(out=rstd[:], in_=mv[:, 1:2], func=AF.Sqrt,
                                 bias=eps_sb[:], scale=1.0)
            nc.vector.reciprocal(out=rstd[:], in_=rstd[:])
            nc.vector.tensor_scalar(out=dst, in0=src, scalar1=mv[:, 0:1],
                                    scalar2=rstd[:], op0=ALU.subtract,
                                    op1=ALU.mult)
            if g_sb is not None:
                nc.vector.tensor_mul(dst, dst, g_sb[:])
                nc.vector.tensor_add(dst, dst, b_sb[:])

        def transpose_into(dstT, src_tile, rt, tp_pool):
            for c in range(DC):
                tp = tp_pool.tile([128, 128], F32, tag="tp")
                nc.tensor.transpose(tp[:], src_tile[:, 128 * c:128 * (c + 1)],
                                    ident_sb[:])
                nc.scalar.copy(dstT[:, c, 128 * rt:128 * (rt + 1)], tp[:])

        def proj_headT(dstT, w_inp, bias_sb, scale, nm):
            """dstT[:, oc, :] = ((x @ w) * scale + bias)^T rows 128oc..+128."""
            with tc.tile_pool(name=f"w_{nm}", bufs=1, side="left") as wpl, \
                 tc.tile_pool(name=f"ps_{nm}", bufs=2, space="PSUM") as pps:
                wts = []
                for c in range(DC):
                    wt = wpl.tile([128, D], BF16, tag=f"w{c}",
                                  name=f"w_{nm}_{c}")
                    nc.sync.dma_start(
                        wt[:], w_inp[128 * c:128 * (c + 1), :])
                    wts.append(wt)
                for oc in range(DC):
                    ps = pps.tile([128, R], F32, tag="ps", name=f"ps_{nm}_{oc}")
                    for c in range(DC):
                        nc.tensor.matmul(
                            ps[:], wts[c][:, 128 * oc:128 * (oc + 1)],
                            xT[:, c, :], start=(c == 0), stop=(c == DC - 1))
                    nc.scalar.activation(dstT[:, oc, :], ps[:],
                                         func=AF.Identity,
                                         bias=bias_sb[:, oc:oc + 1],
                                         scale=scale)

        # DRAM bounce + gathered buffers for the k/v AllGathers
        k_loc = dram.tile([D, R], BF16)
        v_loc = dram.tile([R, VW], BF16)
        k_g = dram.tile([8, D, R], BF16, addr_space="Shared")
        v_g = dram.tile([8, R, VW], BF16, addr_space="Shared")

        es_x = ctx.enter_context(contextlib.ExitStack())      # xT: P0..P3
        es_q = ctx.enter_context(contextlib.ExitStack())      # qT: P3..P4
        xT_pool = es_x.enter_context(
            tc.tile_pool(name="xT_p", bufs=1, side="left"))
        qT_pool = es_q.enter_context(
            tc.tile_pool(name="qT_p", bufs=1, side="right"))
        xT = xT_pool.tile([128, DC, R], BF16)
        qT = qT_pool.tile([128, DC, R], BF16)

        # ================= P0: load + LN1 + transpose ========================
        with tc.tile_pool(name="p0", bufs=2, side="left") as p0, \
             tc.tile_pool(name="p0ps", bufs=4, space="PSUM") as p0ps:
            for rt in range(4):
                hst = p0.tile([128, D], F32, tag="hst")
                nc.sync.dma_start(hst[:], hs[128 * rt:128 * (rt + 1), :])
                xln = p0.tile([128, D], F32, tag="xln")
                layernorm(xln[:], hst[:], p0, ln1_g, ln1_b)
                transpose_into(xT, xln, rt, p0ps)

        # ================= P1: kT; AllGather(k) ==============================
        with tc.tile_pool(name="kT_p", bufs=1, side="right") as kT_pool:
            kT = kT_pool.tile([128, DC, R], BF16)
            proj_headT(kT, io["wk"], bkl_sb, 1.0, "wk")
            k_loc_v = k_loc[:].rearrange("(c p) q -> p c q", p=128)
            for c in range(DC):
                nc.sync.dma_start(k_loc_v[:, c, :], kT[:, c, :].bitcast(BF16))
        nc.gpsimd.collective_compute(
            "AllGather", ALU.bypass,
            replica_groups=[[0, 1, 2, 3, 4, 5, 6, 7]],
            ins=[k_loc.opt()], outs=[k_g.opt()])

        # ================= P2: v rows + ones cols; AllGather(v) ==============
        with tc.tile_pool(name="vaug_p", bufs=1, side="right") as vaug_pool, \
             tc.tile_pool(name="w_wv", bufs=1, side="left") as wpl, \
             tc.tile_pool(name="ps_wv", bufs=2, space="PSUM") as pps:
            vaug = vaug_pool.tile([128, 4, VW], BF16)
            wts = []
            for c in range(DC):
                wt = wpl.tile([128, D], BF16, tag=f"w{c}", name=f"w_wv_{c}")
                nc.sync.dma_start(
                    wt[:], io["wv"][128 * c:128 * (c + 1), :])
                wts.append(wt)
            for pt in range(4):
                for cg in range(2):
                    ps = pps.tile([128, 512], F32, tag="ps",
                                  name=f"ps_wv_{pt}_{cg}")
                    for c in range(DC):
                        nc.tensor.matmul(
                            ps[:], xT[:, c, 128 * pt:128 * (pt + 1)],
                            wts[c][:, 512 * cg:512 * (cg + 1)],
                            start=(c == 0), stop=(c == DC - 1))
                    for hh in range(8):
                        h = 8 * cg + hh
                        nc.scalar.copy(
                            vaug[:, pt, (HD + 1) * h:(HD + 1) * h + HD],
                            ps[:, HD * hh:HD * (hh + 1)])
                nc.sync.dma_start(
                    vaug[:, pt, :].rearrange("p (h e) -> p h e", e=HD + 1)
                    [:, :, HD:HD + 1],
                    io["vones"][:])
                nc.sync.dma_start(v_loc[128 * pt:128 * (pt + 1), :],
                                  vaug[:, pt, :])
        nc.gpsimd.collective_compute(
            "AllGather", ALU.bypass,
            replica_groups=[[0, 1, 2, 3, 4, 5, 6, 7]],
            ins=[v_loc.opt()], outs=[v_g.opt()])

        # ================= P3: qT (scaled by 1/8) ============================
        proj_headT(qT, io["wq"], bq8_sb, 0.125, "wq")
        es_x.close()  # xT no longer needed
        assert qT is not None

        # ================= P4: attention =====================================
        es_attn = ctx.enter_context(contextlib.ExitStack())   # attn_oT: P4..P5
        ao_pool = es_attn.enter_context(tc.tile_pool(name="ao_p", bufs=1, side="left"))
        attn_oT = [ao_pool.tile([128, DC, QB], BF16, name=f"attn_oT{b}")
                   for b in range(NBLK)]
        with tc.tile_pool(name="kg_pool", bufs=1, side="left") as kgp, \
             tc.tile_pool(name="vg_pool", bufs=1, side="left") as vgp, \
             tc.tile_pool(name="own_pool", bufs=2, side="left") as ownp, \
             tc.tile_pool(name="at_sb", bufs=4, side="left") as asb, \
             tc.tile_pool(name="at_norm", bufs=2, side="left") as anorm, \
             tc.tile_pool(name="sc_ps", bufs=2, space="PSUM") as scps, \
             tc.tile_pool(name="oT_ps", bufs=4, space="PSUM") as otps:
            kranks, vranks = [], []
            for r in range(8):
                kr = kgp.tile([128, DC, R], BF16, tag=f"kr{r}", name=f"kr_{r}")
                nc.sync.dma_start(kr[:],
                                  k_g[r].rearrange("(c p) q -> p c q", p=128))
                kranks.append(kr)
                vr = []
                for pt in range(4):
                    vt = vgp.tile([128, VW], BF16, tag=f"vr{r}_{pt}",
                                  name=f"vr_{r}_{pt}")
                    nc.sync.dma_start(vt[:], v_g[r, 128 * pt:128 * (pt + 1), :])
                    vr.append(vt)
                vranks.append(vr)

            def ktile_ap(b, t, hp, hc):
                r, off = _gtile_src(b, t)
                return kranks[r][hp, hc, off:off + KTW]

            def vtile_ap(b, t, vs):
                r, off = _gtile_src(b, t)
                return vranks[r][off // 128][:, vs]

            k_loc_v2 = k_loc[:].rearrange("(c p) q -> p c q", p=128)
            for b in range(NBLK):
                qs = slice(QB * b, QB * (b + 1))
                kown, vown = [], []
                for i in range(2):
                    sl = 2 * b + i
                    ko = ownp.tile([128, DC, KTW], BF16, tag="kown",
                                   name=f"kown_{b}_{i}")
                    nc.sync.dma_start(
                        ko[:], k_loc_v2[:, :, KTW * sl:KTW * (sl + 1)])
                    kown.append(ko)
                    vo = ownp.tile([128, VW], BF16, tag="vown",
                                   name=f"vown_{b}_{i}")
                    nc.sync.dma_start(
                        vo[:], v_loc[KTW * sl:KTW * (sl + 1), :])
                    vown.append(vo)
                for hh in range(H // 2):
                    h0, h1 = 2 * hh, 2 * hh + 1
                    hps = (slice(0, 64), slice(64, 128))
                    vss = (slice((HD + 1) * h0, (HD + 1) * (h0 + 1)),
                           slice((HD + 1) * h1, (HD + 1) * (h1 + 1)))
                    oTs = [otps.tile([HD + 1, QB], F32, tag="oT",
                                     name=f"oT_{b}_{h0}_{j}")
                           for j in range(2)]
                    # pass B first: own (diagonal) kpos tiles, indep of the AG
                    for i in range(2):
                        sc = scps.tile([128, 2, 512], F32, tag="sc",
                                       name=f"scB_{b}_{hh}_{i}")
                        scv = sc[:, :, 0:QB]
                        for j in range(2):
                            nc.tensor.matmul(sc[:, j, 0:QB],
                                             kown[i][hps[j], hh, :],
                                             qT[hps[j], hh, qs],
                                             start=True, stop=True)
                        m_sb = maskA_sb if i == 0 else maskB_sb
                        nc.vector.tensor_add(
                            scv, scv,
                            m_sb.rearrange("p (a b) -> p a b", a=2))
                        ex = asb.tile([128, 2, QB], BF16, tag="ex",
                                      name=f"exB_{b}_{hh}_{i}")
                        nc.scalar.activation(ex[:], scv, func=AF.Exp,
                                             bias=biasB_sb[:, b, i:i + 1],
                                             scale=1.0)
                        for j in range(2):
                            nc.tensor.matmul(oTs[j][:], vown[i][:, vss[j]],
                                             ex[:, j, :],
                                             start=(i == 0), stop=False)
                    # pass A: full (or padded-out) gathered kpos tiles
                    prev_ex = None
                    for t in range(NB_PAD[b]):
                        sc = scps.tile([128, 2, 512], F32, tag="sc",
                                       name=f"scA_{b}_{hh}_{t}")
                        for j in range(2):
                            nc.tensor.matmul(sc[:, j, 0:QB],
                                             ktile_ap(b, t, hps[j], hh),
                                             qT[hps[j], hh, qs],
                                             start=True, stop=True)
                        ex = asb.tile([128, 2, QB], BF16, tag="ex",
                                      name=f"exA_{b}_{hh}_{t}")
                        nc.scalar.activation(ex[:], sc[:, :, 0:QB],
                                             func=AF.Exp,
                                             bias=biasA_sb[:, b, t:t + 1],
                                             scale=1.0)
                        if prev_ex is not None:
                            pt, pex = prev_ex
                            for j in range(2):
                                nc.tensor.matmul(oTs[j][:],
                                                 vtile_ap(b, pt, vss[j]),
                                                 pex[:, j, :],
                                                 start=False, stop=False)
                        prev_ex = (t, ex)
                    pt, pex = prev_ex
                    for j in range(2):
                        nc.tensor.matmul(oTs[j][:], vtile_ap(b, pt, vss[j]),
                                         pex[:, j, :],
                                         start=False, stop=True)
                    # normalize by the ones-column denominator (psum row HD)
                    for j, h in enumerate((h0, h1)):
                        oT = oTs[j]
                        rec = anorm.tile([1, QB], F32, tag="rec",
                                         name=f"rec_{b}_{h}")
                        nc.vector.reciprocal(rec[:], oT[HD:HD + 1, :])
                        rb = anorm.tile([64, QB], F32, tag="rb",
                                        name=f"rb_{b}_{h}")
                        nc.gpsimd.partition_broadcast(rb[:], rec[:])
                        if j == 0:
                            dst = attn_oT[b][0:HD, hh, :]
                            nc.vector.tensor_mul(dst, oT[0:HD, :], rb[:])
                            if apply_bv:
                                nc.vector.tensor_scalar_add(
                                    dst, dst, bvh_sb[:, h:h + 1])
                        else:
                            tmpn = anorm.tile([64, QB], BF16, tag="tmpn",
                                              name=f"tmpn_{b}_{h}")
                            nc.vector.tensor_mul(tmpn[:], oT[0:HD, :], rb[:])
                            if apply_bv:
                                nc.vector.tensor_scalar_add(
                                    tmpn[:], tmpn[:], bvh_sb[:, h:h + 1])
                            nc.sync.dma_start(attn_oT[b][64:128, hh, :],
                                              tmpn[:])
        es_q.close()  # qT done

        # ================= P5: out-proj + residual ===========================
        es_h = ctx.enter_context(contextlib.ExitStack())      # h_sb: P5..P8
        h_pool = es_h.enter_context(tc.tile_pool(name="h_p", bufs=1, side="right"))
        h_sb = h_pool.tile([128, 4, D], F32)
        with tc.tile_pool(name="w_wp", bufs=1, side="left") as wpl, \
             tc.tile_pool(name="hs2", bufs=2, side="left") as hs2, \
             tc.tile_pool(name="ps_wp", bufs=2, space="PSUM") as pps:
            wts = []
            for c in range(DC):
                wt = wpl.tile([128, D], BF16, tag=f"w{c}", name=f"w_wp_{c}")
                nc.sync.dma_start(
                    wt[:], io["wp"][128 * c:128 * (c + 1), :])
                wts.append(wt)
            for rt in range(4):
                b, qt = rt // 2, rt % 2
                hst = hs2.tile([128, D], F32, tag="hst", name=f"hst_{rt}")
                nc.sync.dma_start(hst[:], hs[128 * rt:128 * (rt + 1), :])
                for cg in range(2):
                    ps = pps.tile([128, 512], F32, tag="ps",
                                  name=f"ps_wp_{rt}_{cg}")
                    for c in range(DC):
                        nc.tensor.matmul(
                            ps[:], attn_oT[b][:, c, 128 * qt:128 * (qt + 1)],
                            wts[c][:, 512 * cg:512 * (cg + 1)],
                            start=(c == 0), stop=False)
                    nc.tensor.matmul(ps[:], ones_r[:],
                                     bpr_sb[:, 512 * cg:512 * (cg + 1)],
                                     start=False, stop=True)
                    nc.vector.tensor_add(h_sb[:, rt, 512 * cg:512 * (cg + 1)],
                                         ps[:], hst[:, 512 * cg:512 * (cg + 1)])
        es_attn.close()  # attn_oT done

        # ================= P6: LN2 + transpose ===============================
        es_mlp = ctx.enter_context(contextlib.ExitStack())    # h2T, gT
        mlp_pool = es_mlp.enter_context(tc.tile_pool(name="mlp_p", bufs=1, side="left"))
        h2T = mlp_pool.tile([128, DC, R], BF16)
        gT = mlp_pool.tile([128, GC, R], BF16)
        with tc.tile_pool(name="p6", bufs=2, side="left") as p6, \
             tc.tile_pool(name="p6ps", bufs=4, space="PSUM") as p6ps:
            for rt in range(4):
                h2 = p6.tile([128, D], F32, tag="h2")
                layernorm(h2[:], h_sb[:, rt, :], p6, ln2_g, ln2_b)
                transpose_into(h2T, h2, rt, p6ps)

        # ================= P7: MLP up + gelu =================================
        with tc.tile_pool(name="w_w1", bufs=3, side="left") as wpl, \
             tc.tile_pool(name="ps_w1", bufs=2, space="PSUM") as pps:
            for gc in range(GC):
                wt = wpl.tile([128, DC, 128], BF16, tag="w1")
                nc.sync.dma_start(wt[:], io["w1"][gc])
                ps = pps.tile([128, R], F32, tag="ps", name=f"ps_w1_{gc}")
                for c in range(DC):
                    nc.tensor.matmul(ps[:], wt[:, c, :], h2T[:, c, :],
                                     start=(c == 0), stop=(c == DC - 1))
                nc.scalar.activation(gT[:, gc, :], ps[:], func=AF.Gelu,
                                     bias=b1l_sb[:, gc:gc + 1], scale=1.0)

        # ================= P8: MLP down + bias + residual ====================
        with tc.tile_pool(name="w_w2", bufs=3, side="left") as wpl, \
             tc.tile_pool(name="o_sb", bufs=2, side="left") as osb, \
             tc.tile_pool(name="o_ps", bufs=1, space="PSUM") as pps:
            psts = [pps.tile([128, 512], F32, tag=f"o{i}", name=f"o_ps_{i}")
                    for i in range(8)]
            for gc in range(GC):
                wt = wpl.tile([128, D], BF16, tag="w2")
                nc.sync.dma_start(
                    wt[:], io["w2"][128 * gc:128 * (gc + 1), :])
                for qt in range(4):
                    for cg in range(2):
                        nc.tensor.matmul(
                            psts[2 * qt + cg][:],
                            gT[:, gc, 128 * qt:128 * (qt + 1)],
                            wt[:, 512 * cg:512 * (cg + 1)],
                            start=(gc == 0), stop=False)
            for qt in range(4):
                ot = osb.tile([128, D], F32, tag="ot", name=f"ot_{qt}")
                for cg in range(2):
                    nc.tensor.matmul(psts[2 * qt + cg][:], ones_r[:],
                                     b2r_sb[:, 512 * cg:512 * (cg + 1)],
                                     start=False, stop=True)
                    nc.vector.tensor_add(ot[:, 512 * cg:512 * (cg + 1)],
                                         psts[2 * qt + cg][:],
                                         h_sb[:, qt, 512 * cg:512 * (cg + 1)])
                nc.sync.dma_start(out[128 * qt:128 * (qt + 1), :], ot[:])


# ---------------------------------------------------------------------------
# Host side
# ---------------------------------------------------------------------------

_CACHE = {}
LAST_RESULT = None  # BassKernelResults of the most recent run (for test.py)


def _get_program(key):
    if key not in _CACHE:
        _CACHE[key] = build_program(*key)
    return _CACHE[key]


def _colzero_bias(kpos):
    return np.where((kpos % JD) == (JD - 1), np.float32(NEG), np.float32(0.0))


def kernel(hidden_states, Wq, bq, Wk, bk, Wv, bv, Wp, bp,
           ln1_g, ln1_b, ln2_g, ln2_b, W1, b1, W2, b2):
    f32 = lambda a: np.ascontiguousarray(np.asarray(a, dtype=np.float32))
    hidden_states = f32(hidden_states)
    Wq, bq, Wk, bk, Wv, bv, Wp, bp = map(f32, (Wq, bq, Wk, bk, Wv, bv, Wp, bp))
    ln1_g, ln1_b, ln2_g, ln2_b = map(f32, (ln1_g, ln1_b, ln2_g, ln2_b))
    W1, b1, W2, b2 = map(f32, (W1, b1, W2, b2))

    apply_bv = bool(np.any(bv != 0.0))
    apply_ln1 = bool(np.any(ln1_g != 1.0) or np.any(ln1_b != 0.0))
    apply_ln2 = bool(np.any(ln2_g != 1.0) or np.any(ln2_b != 0.0))
    nc = _get_program((apply_bv, apply_ln1, apply_ln2))

    chunk_major = lambda v: np.ascontiguousarray(v.reshape(-1, 128).T)
    kp = np.arange(KTW)[:, None]
    iq = np.arange(KTW)[None, :]
    tri = np.where(kp <= iq, np.float32(0.0), np.float32(NEG))
    maskAB = np.zeros((2, 128, 2 * KTW), dtype=np.float32)
    maskAB[0, :, :KTW] = tri
    maskAB[1, :, :KTW] = NEG
    maskAB[1, :, KTW:] = tri

    import ml_dtypes
    bf = lambda a: np.ascontiguousarray(a.astype(ml_dtypes.bfloat16))
    w1x = np.ascontiguousarray(
        W1.reshape(DC, 128, GC, 128).transpose(2, 1, 0, 3))
    shared = dict(wq=bf(Wq), wk=bf(Wk), wv=bf(Wv), wp=bf(Wp), w1=bf(w1x),
                  w2=bf(W2),
                  bq8=chunk_major(bq * 0.125), bkl=chunk_major(bk),
                  bvh=np.ascontiguousarray(bv.reshape(H, HD).T),
                  b1l=chunk_major(b1), bpr=bf(bp.reshape(1, D)),
                  b2r=bf(b2.reshape(1, D)), ln1gb=np.stack([ln1_g, ln1_b]),
                  ln2gb=np.stack([ln2_g, ln2_b]), maskAB=maskAB,
                  ident=np.eye(128, dtype=np.float32),
                  onesr=np.ones((1, 128), dtype=ml_dtypes.bfloat16),
                  vones=np.ones((128, H, 1), dtype=ml_dtypes.bfloat16))

    in_maps, row_map = [], []
    for core in range(NCORE):
        # block0 = (batch 0, position core); block1 = (batch 1, pos 7-core)
        positions = (core, 7 - core)
        rows = [np.arange(QB * pb, QB * (pb + 1)) for pb in positions]
        row_map.append(rows)

        biasA = np.full((128, NBLK, max(NB_PAD)), NEG, dtype=np.float32)
        biasB = np.zeros((128, NBLK, 2), dtype=np.float32)
        for b, pb in enumerate(positions):
            for t in range(NB_PAD[b]):
                if t < 2 * pb:
                    biasA[:, b, t] = _colzero_bias(KTW * t + np.arange(KTW))
            for i in range(2):
                biasB[:, b, i] = _colzero_bias(QB * pb + KTW * i
                                               + np.arange(KTW))

        m = dict(shared)
        m["hs"] = np.ascontiguousarray(
            np.concatenate([hidden_states[0, rows[0], :],
                            hidden_states[1, rows[1], :]]))
        m["biasA"] = np.ascontiguousarray(biasA)
        m["biasB"] = np.ascontiguousarray(biasB)
        in_maps.append(m)

    res = run_bass_kernel_spmd(nc, in_maps, core_ids=list(range(NCORE)))
    global LAST_RESULT
    LAST_RESULT = res

    out_full = np.empty((B, S, D), dtype=np.float32)
    for core in range(NCORE):
        rows = row_map[core]
        o = res.results[core]["out"]
        out_full[0, rows[0], :] = o[:QB]
        out_full[1, rows[1], :] = o[QB:]
    return out_full
